# revision 4
# baseline (speedup 1.0000x reference)
"""Normalized-adjacency kernel (EstimateAdj.normalize, symmetric=False) for TRN2.

out = mx * r_inv[:, None] * r_inv[None, :]   where mx = adj + I,
r_inv = rowsum(mx) ** -0.5.

Strategy (8 NeuronCores, row-sharded, raw Bass with explicit semaphores):
  - host: add 1.0 to the diagonal (O(n)), split rows into 8 shards of 1024
  - device, per core:
      pass 1: stream 8 tiles of [128 x 8192] (SWDGE DMA), DVE rowsum each
      r_inv = 1/sqrt(rowsum)  (ACT sqrt + DVE reciprocal)
      AllGather the 1024 local r_inv values -> full 8192 vector (DRAM)
      broadcast-DMA full r_inv into a [128 x 8192] SBUF tile (column scale)
      pass 2: stream tiles again; one fused DVE scalar_tensor_tensor:
              out = (tile * r_inv_row_scalar) * colscale ; DMA out
  - host: concatenate the 8 output shards

Tile t of a shard holds shard rows [t::T] (partition p <-> shard row p*T + t)
so the SBUF rowsum layout [128, T] maps to a contiguous global-row-major DRAM
vector without any transpose.

Each SBUF slot has its own DMA-completion semaphore so a wait value is only
ever crossed by a single in-flight DMA (per-slot, single outstanding).
"""

import numpy as np

import concourse.bass as bass
import concourse.mybir as mybir
from concourse.bass_utils import run_bass_kernel_spmd

N = 8192
NCORES = 8
SHARD = N // NCORES  # 1024
P = 128
T = SHARD // P  # 8 tiles per core

F32 = mybir.dt.float32
NIN = 3  # input-tile slots
NOUT = 2  # output-tile slots


def build_kernel(n=N, ncores=NCORES):
    shard = n // ncores
    tt = shard // P

    nc = bass.Bass(num_devices=ncores)
    mx = nc.dram_tensor("mx", [shard, n], F32, kind="ExternalInput")
    out = nc.dram_tensor("out", [shard, n], F32, kind="ExternalOutput")
    cc_in = nc.dram_tensor("cc_in", [shard], F32)
    cc_out = nc.dram_tensor("cc_out", [n], F32, addr_space="Shared")

    # shard row = p * tt + t ; tile t = shard rows [t::tt]
    mx_v = mx.rearrange("(p t) n -> t p n", t=tt)
    out_v = out.rearrange("(p t) n -> t p n", t=tt)
    cc_in_v = cc_in.rearrange("(p t) -> p t", t=tt)

    nin = min(NIN, tt)
    nout = min(NOUT, tt)
    npre = nin  # pass-2 loads prefetched before the allgather wait

    # load t of pass q consumes input slot t % nin; in_val[q][t] is the
    # s_in[slot] value that marks that load's completion.
    in_count = [0] * nin
    in_val = [[0] * tt for _ in range(2)]
    for q in range(2):
        for t in range(tt):
            in_count[t % nin] += 16
            in_val[q][t] = in_count[t % nin]
    out_count = [0] * nout
    out_val = [0] * tt
    for t in range(tt):
        out_count[t % nout] += 16
        out_val[t] = out_count[t % nout]

    with (
        nc.sbuf_tensor("in0", [P, n], F32) as in0,
        nc.sbuf_tensor("in1", [P, n], F32) as in1,
        nc.sbuf_tensor("in2", [P, n], F32) as in2,
        nc.sbuf_tensor("ot0", [P, n], F32) as ot0,
        nc.sbuf_tensor("ot1", [P, n], F32) as ot1,
        nc.sbuf_tensor("colscale", [P, n], F32) as colscale,
        nc.sbuf_tensor("rs", [P, tt], F32) as rs,
        nc.sbuf_tensor("rinv", [P, tt], F32) as rinv,
        nc.semaphore("s_in0") as s_in0,  # per-input-slot loads, +16
        nc.semaphore("s_in1") as s_in1,
        nc.semaphore("s_in2") as s_in2,
        nc.semaphore("s_out0") as s_out0,  # per-output-slot stores, +16
        nc.semaphore("s_out1") as s_out1,
        nc.semaphore("s_red") as s_red,  # reduces, +1 each
        nc.semaphore("s_sqrt") as s_sqrt,  # +1
        nc.semaphore("s_rcp") as s_rcp,  # +1
        nc.semaphore("s_ccin") as s_ccin,  # +16
        nc.semaphore("s_cc") as s_cc,  # allgather, +1
        nc.semaphore("s_cs") as s_cs,  # colscale bcast, +16
        nc.semaphore("s_stt") as s_stt,  # fused scales, +1 each
        nc.Block() as block,
    ):
        ins = [in0, in1, in2][:nin]
        ots = [ot0, ot1][:nout]
        s_in = [s_in0, s_in1, s_in2][:nin]
        s_out = [s_out0, s_out1][:nout]

        @block.gpsimd
        def _(g):
            # pass 1 loads
            for t in range(tt):
                if t >= nin:
                    g.wait_ge(s_red, t - nin + 1)  # slot's reduce done
                g.dma_start(ins[t % nin][:, :], mx_v[t]).then_inc(s_in[t % nin], 16)

            # local r_inv -> DRAM, allgather
            g.wait_ge(s_rcp, 1)
            g.dma_start(cc_in_v, rinv[:, :]).then_inc(s_ccin, 16)
            g.wait_ge(s_ccin, 16)
            g.collective_compute(
                "AllGather",
                mybir.AluOpType.bypass,
                replica_groups=[list(range(ncores))],
                ins=[cc_in[:]],
                outs=[cc_out[:]],
            ).then_inc(s_cc, 1)

            # prefetch pass-2 loads while the allgather is in flight
            g.wait_ge(s_red, tt)  # all pass-1 reduces done -> slots free
            for t in range(npre):
                g.dma_start(ins[t % nin][:, :], mx_v[t]).then_inc(s_in[t % nin], 16)

            # column-scale broadcast (after allgather)
            g.wait_ge(s_cc, 1)
            g.dma_start(
                colscale[:, :], cc_out[:].partition_broadcast(P)
            ).then_inc(s_cs, 16)

            # pass 2 steady state
            for t in range(tt):
                if t >= npre:
                    g.wait_ge(s_stt, t - nin + 1)  # slot's scale done
                    g.dma_start(ins[t % nin][:, :], mx_v[t]).then_inc(
                        s_in[t % nin], 16
                    )
                g.wait_ge(s_stt, t + 1)
                g.dma_start(out_v[t], ots[t % nout][:, :]).then_inc(
                    s_out[t % nout], 16
                )

            # all stores landed before halt
            for j in range(nout):
                g.wait_ge(s_out[j], out_count[j])

        @block.vector
        def _(v):
            # pass 1: rowsums
            for t in range(tt):
                v.wait_ge(s_in[t % nin], in_val[0][t])
                v.reduce_sum(
                    rs[:, t : t + 1], ins[t % nin][:, :], axis=mybir.AxisListType.X
                ).then_inc(s_red, 1)
            # r_inv = 1/sqrt(rowsum): ACT did sqrt, finish with reciprocal
            v.wait_ge(s_sqrt, 1)
            v.reciprocal(rinv[:, :], rinv[:, :]).then_inc(s_rcp, 1)
            # pass 2: fused row+column scale
            v.wait_ge(s_cs, 16)
            for t in range(tt):
                v.wait_ge(s_in[t % nin], in_val[1][t])
                if t >= nout:
                    # out slot free (its previous store completed)
                    v.wait_ge(s_out[t % nout], out_val[t] - 16)
                v.scalar_tensor_tensor(
                    ots[t % nout][:, :],
                    ins[t % nin][:, :],
                    rinv[:, t : t + 1],
                    colscale[:, :],
                    op0=mybir.AluOpType.mult,
                    op1=mybir.AluOpType.mult,
                ).then_inc(s_stt, 1)

        @block.scalar
        def _(s):
            s.wait_ge(s_red, tt)
            s.sqrt(rinv[:, :], rs[:, :]).then_inc(s_sqrt, 1)

    return nc


_NC_CACHE = {}


def _get_nc(n=N, ncores=NCORES):
    key = (n, ncores)
    if key not in _NC_CACHE:
        _NC_CACHE[key] = build_kernel(n, ncores)
    return _NC_CACHE[key]


def kernel(adj, **run_kwargs):
    adj = np.asarray(adj)
    assert adj.shape == (N, N) and adj.dtype == np.float32
    mx = adj.copy()
    idx = np.arange(N)
    mx[idx, idx] += 1.0

    in_maps = [{"mx": mx[c * SHARD : (c + 1) * SHARD]} for c in range(NCORES)]
    nc = _get_nc()
    res = run_bass_kernel_spmd(nc, in_maps, list(range(NCORES)), **run_kwargs)
    out = np.concatenate([res.results[c]["out"] for c in range(NCORES)], axis=0)
    if run_kwargs:
        return out, res
    return out


# revision 7
# speedup vs baseline: 1.2064x; 1.2064x over previous
"""Normalized-adjacency kernel (EstimateAdj.normalize, symmetric=False) for TRN2.

out = mx * r_inv[:, None] * r_inv[None, :]   where mx = adj + I,
r_inv = rowsum(mx) ** -0.5.

Strategy (8 NeuronCores, row-sharded, raw Bass with explicit semaphores):
  - host: add 1.0 to the diagonal (O(n)), split rows into 8 shards of 1024
  - device, per core (tiles of [128 x n], shard row = p*T + t):
      pass 1: stream the first tiles through 2 SBUF slots, keep the last 3
              resident (cached); DVE rowsum each tile
      r_inv = 1/sqrt(rowsum)  (ACT sqrt + DVE reciprocal)
      AllGather the local r_inv (DRAM) -> full n vector
      broadcast-DMA full r_inv into a [128 x n] SBUF tile (column scale)
      pass 2: cached tiles are scaled in place immediately; streamed tiles are
              re-loaded; one fused DVE scalar_tensor_tensor per tile:
              tile = (tile * r_inv_row_scalar) * colscale ; store
  - engines: gpsimd/Pool = loads + allgather; SP/sync = stores + small DMAs;
    DVE = reduces + fused scales; ACT = sqrt.  Loads and stores ride separate
    DMA rings so neither blocks the other.
  - host: concatenate the 8 output shards

Every semaphore has at most one in-flight DMA whenever a wait threshold is
crossed (per-slot semaphores), which the CoreSim race detector requires.
"""

import numpy as np

import concourse.bass as bass
import concourse.mybir as mybir
from concourse.bass_utils import run_bass_kernel_spmd

N = 8192
NCORES = 8
SHARD = N // NCORES  # 1024
P = 128
T = SHARD // P  # 8 tiles per core

F32 = mybir.dt.float32
NSTREAM = 2  # streaming tile slots
NCACHE = 3  # pass-1-resident tile slots (skip their pass-2 reload)


def build_kernel(n=N, ncores=NCORES):
    shard = n // ncores
    tt = shard // P

    ncache = min(NCACHE, max(tt - NSTREAM, 0))
    nstream = min(NSTREAM, tt - ncache)
    stream_tiles = list(range(tt - ncache))
    cached_tiles = list(range(tt - ncache, tt))

    def slot_of(t):
        if t in cached_tiles:
            return nstream + (t - (tt - ncache))
        return t % nstream

    # pass-2 processing order: interleave cached (ready instantly) w/ streamed
    order = []
    i = j = 0
    while i < len(stream_tiles) or j < len(cached_tiles):
        if j < len(cached_tiles):
            order.append(cached_tiles[j])
            j += 1
        if i < len(stream_tiles):
            order.append(stream_tiles[i])
            i += 1

    # per-slot cumulative load-completion values (s_in[slot])
    nslots = nstream + ncache
    in_count = [0] * nslots
    in_val1 = [0] * tt
    for t in range(tt):
        in_count[slot_of(t)] += 16
        in_val1[t] = in_count[slot_of(t)]
    in_val2 = {}
    for t in stream_tiles:
        in_count[slot_of(t)] += 16
        in_val2[t] = in_count[slot_of(t)]

    # per-stream-slot cumulative store-completion values (s_souts[slot])
    souts_count = [0] * max(nstream, 1)
    souts_val = {}
    for t in stream_tiles:
        souts_count[slot_of(t)] += 16
        souts_val[t] = souts_count[slot_of(t)]

    nc = bass.Bass(num_devices=ncores)
    mx = nc.dram_tensor("mx", [shard, n], F32, kind="ExternalInput")
    out = nc.dram_tensor("out", [shard, n], F32, kind="ExternalOutput")
    cc_in = nc.dram_tensor("cc_in", [shard], F32)
    cc_out = nc.dram_tensor("cc_out", [n], F32, addr_space="Shared")

    mx_v = mx.rearrange("(p t) n -> t p n", t=tt)
    out_v = out.rearrange("(p t) n -> t p n", t=tt)
    cc_in_v = cc_in.rearrange("(p t) -> p t", t=tt)

    from contextlib import ExitStack

    with ExitStack() as ctx:
        slots = [
            ctx.enter_context(nc.sbuf_tensor(f"tile{i}", [P, n], F32))
            for i in range(nslots)
        ]
        colscale = ctx.enter_context(nc.sbuf_tensor("colscale", [P, n], F32))
        rs = ctx.enter_context(nc.sbuf_tensor("rs", [P, tt], F32))
        rinv = ctx.enter_context(nc.sbuf_tensor("rinv", [P, tt], F32))

        # per-slot loads +16; per-slot stores +16; compute sems +1
        s_in = [
            ctx.enter_context(nc.semaphore(f"s_in{i}")) for i in range(nslots)
        ]
        s_souts = [
            ctx.enter_context(nc.semaphore(f"s_souts{i}"))
            for i in range(max(nstream, 1))
        ]
        s_soutc = [
            ctx.enter_context(nc.semaphore(f"s_soutc{i}"))
            for i in range(max(ncache, 1))
        ]
        s_red = ctx.enter_context(nc.semaphore("s_red"))
        s_sqrt = ctx.enter_context(nc.semaphore("s_sqrt"))
        s_rcp = ctx.enter_context(nc.semaphore("s_rcp"))
        s_ccin = ctx.enter_context(nc.semaphore("s_ccin"))
        s_cc = ctx.enter_context(nc.semaphore("s_cc"))
        s_cs = ctx.enter_context(nc.semaphore("s_cs"))
        s_stt = ctx.enter_context(nc.semaphore("s_stt"))
        block = ctx.enter_context(nc.Block())

        @block.gpsimd
        def _(g):
            # pass 1 loads
            for t in range(tt):
                if t in stream_tiles and t >= nstream:
                    g.wait_ge(s_red, t - nstream + 1)  # slot's reduce done
                g.dma_start(slots[slot_of(t)][:, :], mx_v[t]).then_inc(
                    s_in[slot_of(t)], 16
                )

            # prefetch the first pass-2 stream loads; then allgather
            if stream_tiles:
                g.wait_ge(s_red, len(stream_tiles))  # stream slots all free
            for t in stream_tiles[:nstream]:
                g.dma_start(slots[slot_of(t)][:, :], mx_v[t]).then_inc(
                    s_in[slot_of(t)], 16
                )
            g.wait_ge(s_ccin, 16)  # SP wrote local r_inv to DRAM
            g.collective_compute(
                "AllGather",
                mybir.AluOpType.bypass,
                replica_groups=[list(range(ncores))],
                ins=[cc_in[:]],
                outs=[cc_out[:]],
            ).then_inc(s_cc, 1)

            # remaining pass-2 stream loads (slot free when its store landed)
            for t in stream_tiles[nstream:]:
                g.wait_ge(s_souts[slot_of(t)], souts_val[t] - 16)
                g.dma_start(slots[slot_of(t)][:, :], mx_v[t]).then_inc(
                    s_in[slot_of(t)], 16
                )

        @block.sync
        def _(sp):
            # local r_inv -> DRAM (global-row-major)
            sp.wait_ge(s_rcp, 1)
            sp.dma_start(cc_in_v, rinv[:, :]).then_inc(s_ccin, 16)
            # column-scale broadcast once the allgather lands
            sp.wait_ge(s_cc, 1)
            sp.dma_start(
                colscale[:, :], cc_out[:].partition_broadcast(P)
            ).then_inc(s_cs, 16)
            # stores, in pass-2 processing order
            for k, t in enumerate(order):
                sp.wait_ge(s_stt, k + 1)
                if t in stream_tiles:
                    if souts_val[t] > 16:
                        sp.wait_ge(s_souts[slot_of(t)], souts_val[t] - 16)
                    sem, val = s_souts[slot_of(t)], 16
                else:
                    sem, val = s_soutc[slot_of(t) - nstream], 16
                sp.dma_start(out_v[t], slots[slot_of(t)][:, :]).then_inc(sem, 16)
            # all stores landed before halt
            for s_idx in range(nstream):
                sp.wait_ge(s_souts[s_idx], souts_count[s_idx])
            for c_idx in range(ncache):
                sp.wait_ge(s_soutc[c_idx], 16)

        @block.vector
        def _(v):
            # pass 1: rowsums
            for t in range(tt):
                v.wait_ge(s_in[slot_of(t)], in_val1[t])
                v.reduce_sum(
                    rs[:, t : t + 1],
                    slots[slot_of(t)][:, :],
                    axis=mybir.AxisListType.X,
                ).then_inc(s_red, 1)
            # r_inv = 1/sqrt(rowsum): ACT did sqrt, finish with reciprocal
            v.wait_ge(s_sqrt, 1)
            v.reciprocal(rinv[:, :], rinv[:, :]).then_inc(s_rcp, 1)
            # pass 2: fused row+column scale, in place
            v.wait_ge(s_cs, 16)
            for t in order:
                if t in stream_tiles:
                    v.wait_ge(s_in[slot_of(t)], in_val2[t])
                v.scalar_tensor_tensor(
                    slots[slot_of(t)][:, :],
                    slots[slot_of(t)][:, :],
                    rinv[:, t : t + 1],
                    colscale[:, :],
                    op0=mybir.AluOpType.mult,
                    op1=mybir.AluOpType.mult,
                ).then_inc(s_stt, 1)

        @block.scalar
        def _(s):
            s.wait_ge(s_red, tt)
            s.sqrt(rinv[:, :], rs[:, :]).then_inc(s_sqrt, 1)

    return nc


_NC_CACHE = {}


def _get_nc(n=N, ncores=NCORES):
    key = (n, ncores)
    if key not in _NC_CACHE:
        _NC_CACHE[key] = build_kernel(n, ncores)
    return _NC_CACHE[key]


def kernel(adj, **run_kwargs):
    adj = np.asarray(adj)
    assert adj.shape == (N, N) and adj.dtype == np.float32
    mx = adj.copy()
    idx = np.arange(N)
    mx[idx, idx] += 1.0

    in_maps = [{"mx": mx[c * SHARD : (c + 1) * SHARD]} for c in range(NCORES)]
    nc = _get_nc()
    res = run_bass_kernel_spmd(nc, in_maps, list(range(NCORES)), **run_kwargs)
    out = np.concatenate([res.results[c]["out"] for c in range(NCORES)], axis=0)
    if run_kwargs:
        return out, res
    return out


# revision 12
# speedup vs baseline: 1.2383x; 1.0265x over previous
"""Normalized-adjacency kernel (EstimateAdj.normalize, symmetric=False) for TRN2.

out = mx * r_inv[:, None] * r_inv[None, :]   where mx = adj + I,
r_inv = rowsum(mx) ** -0.5.

Strategy (8 NeuronCores, row-sharded, raw Bass with explicit semaphores):
  - host: add 1.0 to the diagonal (O(n)), split rows into 8 shards
  - device, per core: work items are HALF-tiles [128 x n/2]
    (tile t = shard rows [t::T], halves h=0/1 split the columns):
      pass 1: stream the first 11 halves through 5 SBUF slots, keep the last
              5 halves resident; DVE partial rowsum per half, then combine
      r_inv = 1/sqrt(rowsum)  (ACT sqrt + DVE reciprocal)
      AllGather local r_inv (DRAM) -> full n vector; while it is in flight,
      the 5 stream slots prefetch the first 5 pass-2 halves (~10 MiB) so the
      DMA rings stay busy through the collective's ~30 us latency
      pass 2: fused in-place DVE scalar_tensor_tensor per half:
              half = (half * r_inv_row_scalar) * colscale[:, h-slice]; store
  - engines: gpsimd/Pool = loads + allgather; SP/sync = stores + small DMAs;
    DVE = reduces + fused scales; ACT = sqrt.  Loads and stores ride separate
    DMA rings so neither blocks the other.
  - host: concatenate the 8 output shards

Every semaphore wait threshold is crossed by a single in-flight DMA
(per-slot semaphores), which the CoreSim race detector requires.
"""

from contextlib import ExitStack

import numpy as np

import concourse.bass as bass
import concourse.mybir as mybir
from concourse.bass_utils import run_bass_kernel_spmd

N = 8192
NCORES = 8
SHARD = N // NCORES  # 1024
P = 128
T = SHARD // P  # 8 tiles per core
H = 2  # column halves per tile

F32 = mybir.dt.float32
NSTREAM = 5  # streaming half-tile slots
NCACHE = 5  # pass-1-resident half-tile slots


def build_kernel(n=N, ncores=NCORES):
    shard = n // ncores
    tt = shard // P
    w = n // H  # half width
    items = [(t, h) for t in range(tt) for h in range(H)]  # load order
    ni = len(items)

    ncache = min(NCACHE, max(ni - NSTREAM, 0))
    nstream = min(NSTREAM, ni - ncache)
    stream_items = list(range(ni - ncache))  # indices into `items`
    cached_items = list(range(ni - ncache, ni))

    def slot_of(i):
        if i >= ni - ncache:
            return nstream + (i - (ni - ncache))
        return i % nstream

    # pass-2 processing order: interleave cached (ready instantly) w/ streamed
    order = []
    a = b = 0
    while a < len(stream_items) or b < len(cached_items):
        if b < len(cached_items):
            order.append(cached_items[b])
            b += 1
        if a < len(stream_items):
            order.append(stream_items[a])
            a += 1

    # per-slot cumulative load-completion values (s_in[slot])
    nslots = nstream + ncache
    in_count = [0] * nslots
    in_val1 = [0] * ni
    for i in range(ni):
        in_count[slot_of(i)] += 16
        in_val1[i] = in_count[slot_of(i)]
    in_val2 = {}
    for i in stream_items:
        in_count[slot_of(i)] += 16
        in_val2[i] = in_count[slot_of(i)]

    # per-stream-slot cumulative store-completion values (s_souts[slot])
    souts_count = [0] * max(nstream, 1)
    souts_val = {}
    for i in stream_items:
        souts_count[slot_of(i)] += 16
        souts_val[i] = souts_count[slot_of(i)]

    nc = bass.Bass(num_devices=ncores)
    mx = nc.dram_tensor("mx", [shard, n], F32, kind="ExternalInput")
    out = nc.dram_tensor("out", [shard, n], F32, kind="ExternalOutput")
    cc_in = nc.dram_tensor("cc_in", [shard], F32)
    cc_out = nc.dram_tensor("cc_out", [n], F32, addr_space="Shared")

    # [tt, 128, H, w]: tile t, partition p, half h -> shard row p*tt + t
    mx_v = mx.rearrange("(p t) (h w) -> t p h w", t=tt, h=H)
    out_v = out.rearrange("(p t) (h w) -> t p h w", t=tt, h=H)
    cc_in_v = cc_in.rearrange("(p t) -> p t", t=tt)

    with ExitStack() as ctx:
        slots = [
            ctx.enter_context(nc.sbuf_tensor(f"tile{i}", [P, w], F32))
            for i in range(nslots)
        ]
        colscale = ctx.enter_context(nc.sbuf_tensor("colscale", [P, n], F32))
        ps = ctx.enter_context(nc.sbuf_tensor("ps", [P, ni], F32))
        rs = ctx.enter_context(nc.sbuf_tensor("rs", [P, tt], F32))
        rinv = ctx.enter_context(nc.sbuf_tensor("rinv", [P, tt], F32))

        # per-slot loads +16; per-stream-slot stores +16; compute sems +1
        s_in = [
            ctx.enter_context(nc.semaphore(f"s_in{i}")) for i in range(nslots)
        ]
        s_souts = [
            ctx.enter_context(nc.semaphore(f"s_souts{i}"))
            for i in range(max(nstream, 1))
        ]
        s_soutc = ctx.enter_context(nc.semaphore("s_soutc"))  # cached stores
        s_red = ctx.enter_context(nc.semaphore("s_red"))
        s_sqrt = ctx.enter_context(nc.semaphore("s_sqrt"))
        s_rcp = ctx.enter_context(nc.semaphore("s_rcp"))
        s_ccin = ctx.enter_context(nc.semaphore("s_ccin"))
        s_cc = ctx.enter_context(nc.semaphore("s_cc"))
        s_cs = ctx.enter_context(nc.semaphore("s_cs"))
        s_stt = ctx.enter_context(nc.semaphore("s_stt"))
        block = ctx.enter_context(nc.Block())

        def item_src(i):
            t, h = items[i]
            return mx_v[t, :, h]

        def item_dst(i):
            t, h = items[i]
            return out_v[t, :, h]

        @block.gpsimd
        def _(g):
            # pass 1 loads
            for i in range(ni):
                if i in stream_items and i >= nstream:
                    g.wait_ge(s_red, i - nstream + 1)  # slot's reduce done
                g.dma_start(slots[slot_of(i)][:, :], item_src(i)).then_inc(
                    s_in[slot_of(i)], 16
                )

            # prefetch the first pass-2 stream loads (fills the AG window)
            if stream_items:
                g.wait_ge(s_red, len(stream_items))  # stream slots all free
            for i in stream_items[:nstream]:
                g.dma_start(slots[slot_of(i)][:, :], item_src(i)).then_inc(
                    s_in[slot_of(i)], 16
                )

            g.wait_ge(s_ccin, 16)  # SP wrote local r_inv to DRAM
            g.collective_compute(
                "AllGather",
                mybir.AluOpType.bypass,
                replica_groups=[list(range(ncores))],
                ins=[cc_in[:]],
                outs=[cc_out[:]],
            ).then_inc(s_cc, 1)

            # remaining pass-2 stream loads (slot free when its store landed)
            for i in stream_items[nstream:]:
                g.wait_ge(s_souts[slot_of(i)], souts_val[i] - 16)
                g.dma_start(slots[slot_of(i)][:, :], item_src(i)).then_inc(
                    s_in[slot_of(i)], 16
                )

        @block.sync
        def _(sp):
            # local r_inv -> DRAM (global-row-major)
            sp.wait_ge(s_rcp, 1)
            sp.dma_start(cc_in_v, rinv[:, :]).then_inc(s_ccin, 16)
            # column-scale broadcast once the allgather lands
            sp.wait_ge(s_cc, 1)
            sp.dma_start(
                colscale[:, :], cc_out[:].partition_broadcast(P)
            ).then_inc(s_cs, 16)
            # stores, in pass-2 processing order
            for k, i in enumerate(order):
                sp.wait_ge(s_stt, k + 1)
                if i in in_val2:  # streamed
                    if souts_val[i] > 16:
                        sp.wait_ge(s_souts[slot_of(i)], souts_val[i] - 16)
                    sem = s_souts[slot_of(i)]
                else:
                    sem = s_soutc
                sp.dma_start(item_dst(i), slots[slot_of(i)][:, :]).then_inc(
                    sem, 16
                )
            # all stores landed before halt
            for s_idx in range(nstream):
                sp.wait_ge(s_souts[s_idx], souts_count[s_idx])
            if ncache:
                sp.wait_ge(s_soutc, 16 * ncache)

        @block.vector
        def _(v):
            # pass 1: partial rowsums per half
            for i in range(ni):
                v.wait_ge(s_in[slot_of(i)], in_val1[i])
                v.reduce_sum(
                    ps[:, i : i + 1],
                    slots[slot_of(i)][:, :],
                    axis=mybir.AxisListType.X,
                ).then_inc(s_red, 1)
            # combine halves: rs[:, t] = sum_h ps[:, t*H + h]
            # (self-wait: DVE pipeline must drain the reduces' writebacks)
            v.wait_ge(s_red, ni)
            if H == 2:
                v.scalar_tensor_tensor(
                    rs[:, :],
                    ps[:, 0::2],
                    1.0,
                    ps[:, 1::2],
                    op0=mybir.AluOpType.mult,
                    op1=mybir.AluOpType.add,
                ).then_inc(s_red, 1)
            else:
                raise NotImplementedError
            # r_inv = 1/sqrt(rowsum): ACT does sqrt, finish with reciprocal
            v.wait_ge(s_sqrt, 1)
            v.reciprocal(rinv[:, :], rinv[:, :]).then_inc(s_rcp, 1)
            # pass 2: fused row+column scale, in place
            # (self-wait drains the reciprocal's writeback before stts read rinv)
            v.wait_ge(s_rcp, 1)
            v.wait_ge(s_cs, 16)
            for i in order:
                t, h = items[i]
                if i in in_val2:  # streamed: wait for its pass-2 load
                    v.wait_ge(s_in[slot_of(i)], in_val2[i])
                v.scalar_tensor_tensor(
                    slots[slot_of(i)][:, :],
                    slots[slot_of(i)][:, :],
                    rinv[:, t : t + 1],
                    colscale[:, h * w : (h + 1) * w],
                    op0=mybir.AluOpType.mult,
                    op1=mybir.AluOpType.mult,
                ).then_inc(s_stt, 1)

        @block.scalar
        def _(s):
            s.wait_ge(s_red, ni + 1)  # all reduces + the half-combine
            s.sqrt(rinv[:, :], rs[:, :]).then_inc(s_sqrt, 1)

    return nc


_NC_CACHE = {}


def _get_nc(n=N, ncores=NCORES):
    key = (n, ncores)
    if key not in _NC_CACHE:
        _NC_CACHE[key] = build_kernel(n, ncores)
    return _NC_CACHE[key]


def kernel(adj, **run_kwargs):
    adj = np.asarray(adj)
    assert adj.shape == (N, N) and adj.dtype == np.float32
    mx = adj.copy()
    idx = np.arange(N)
    mx[idx, idx] += 1.0

    in_maps = [{"mx": mx[c * SHARD : (c + 1) * SHARD]} for c in range(NCORES)]
    nc = _get_nc()
    res = run_bass_kernel_spmd(nc, in_maps, list(range(NCORES)), **run_kwargs)
    out = np.concatenate([res.results[c]["out"] for c in range(NCORES)], axis=0)
    if run_kwargs:
        return out, res
    return out


# revision 15
# speedup vs baseline: 1.3137x; 1.0609x over previous
"""Normalized-adjacency kernel (EstimateAdj.normalize, symmetric=False) for TRN2.

out = mx * r_inv[:, None] * r_inv[None, :]   where mx = adj + I,
r_inv = rowsum(mx) ** -0.5.

Strategy (8 NeuronCores, row-sharded, raw Bass with explicit semaphores):
  - host: add 1.0 to the diagonal (O(n)), split rows into 8 shards
  - device, per core: work items are HALF-tiles [128 x n/2]
    (tile t = shard rows [t*128:(t+1)*128], halves h split the columns):
      pass 1: stream the first 11 halves through 5 SBUF slots, keep the last
              5 halves resident.  Rowsums run on the SCALAR engine
              (activation Copy with accum_out), so the DVE stays free and the
              loads, not the reduces, pace the pass.
      r_inv = 1/sqrt(rowsum) (ACT sqrt + DVE reciprocal); PE transposes
      r_inv via an identity matmul so the DRAM write of the local r_inv is
      8 contiguous 512B descriptors instead of 128 scattered 32B ones.
      AllGather local r_inv (DRAM) -> full n vector; while it is in flight
      the 5 stream slots prefetch the first 5 pass-2 halves (~10 MiB).
      pass 2: fused in-place DVE scalar_tensor_tensor per half:
              half = (half * r_inv_row_scalar) * colscale[:, h-slice]; store.
              Prefetched stream halves are processed FIRST so their stores
              complete early and un-gate the remaining reloads (the reload
              chain is bandwidth-bound, not latency-bound).
  - engines: gpsimd/Pool = loads + allgather; SP/sync = stores + small DMAs;
    DVE = fused scales; ACT = rowsums + sqrt; PE = r_inv transpose.
  - host: concatenate the 8 output shards
"""

from contextlib import ExitStack

import numpy as np

import concourse.bass as bass
import concourse.mybir as mybir
from concourse.bass_utils import run_bass_kernel_spmd

N = 8192
NCORES = 8
SHARD = N // NCORES  # 1024
P = 128
T = SHARD // P  # 8 tiles per core
H = 2  # column halves per tile

F32 = mybir.dt.float32
NSTREAM = 5  # streaming half-tile slots
NCACHE = 5  # pass-1-resident half-tile slots


def build_kernel(n=N, ncores=NCORES):
    shard = n // ncores
    tt = shard // P
    w = n // H  # half width
    items = [(t, h) for t in range(tt) for h in range(H)]  # load order
    ni = len(items)

    ncache = min(NCACHE, max(ni - NSTREAM, 0))
    nstream = min(NSTREAM, ni - ncache)
    stream_items = list(range(ni - ncache))  # indices into `items`
    cached_items = list(range(ni - ncache, ni))

    def slot_of(i):
        if i >= ni - ncache:
            return nstream + (i - (ni - ncache))
        return i % nstream

    # pass-2 order: prefetched stream halves first (their stores un-gate the
    # reloads), then cached halves, then the reloaded stream halves.
    order = (
        stream_items[:nstream] + cached_items + stream_items[nstream:]
    )

    # per-slot cumulative load-completion values (s_in[slot])
    nslots = nstream + ncache
    in_count = [0] * nslots
    in_val1 = [0] * ni
    for i in range(ni):
        in_count[slot_of(i)] += 16
        in_val1[i] = in_count[slot_of(i)]
    in_val2 = {}
    for i in stream_items:
        in_count[slot_of(i)] += 16
        in_val2[i] = in_count[slot_of(i)]

    # per-stream-slot cumulative store-completion values (s_souts[slot])
    souts_count = [0] * max(nstream, 1)
    souts_val = {}
    for i in stream_items:
        souts_count[slot_of(i)] += 16
        souts_val[i] = souts_count[slot_of(i)]

    nc = bass.Bass(num_devices=ncores)
    mx = nc.dram_tensor("mx", [shard, n], F32, kind="ExternalInput")
    eye = nc.dram_tensor("eye", [P, P], F32, kind="ExternalInput")
    out = nc.dram_tensor("out", [shard, n], F32, kind="ExternalOutput")
    cc_in = nc.dram_tensor("cc_in", [shard], F32)
    cc_out = nc.dram_tensor("cc_out", [n], F32, addr_space="Shared")

    # blocked tiling: tile t, partition p, half h -> shard row t*128 + p
    mx_v = mx.rearrange("(t p) (h w) -> t p h w", p=P, h=H)
    out_v = out.rearrange("(t p) (h w) -> t p h w", p=P, h=H)

    with ExitStack() as ctx:
        slots = [
            ctx.enter_context(nc.sbuf_tensor(f"tile{i}", [P, w], F32))
            for i in range(nslots)
        ]
        colscale = ctx.enter_context(nc.sbuf_tensor("colscale", [P, n], F32))
        eye_sb = ctx.enter_context(nc.sbuf_tensor("eye_sb", [P, P], F32))
        ps = ctx.enter_context(nc.sbuf_tensor("ps", [P, ni], F32))
        rs = ctx.enter_context(nc.sbuf_tensor("rs", [P, tt], F32))
        rinv = ctx.enter_context(nc.sbuf_tensor("rinv", [P, tt], F32))
        ptc = ctx.enter_context(nc.sbuf_tensor("ptc", [tt, P], F32))
        pt = ctx.enter_context(nc.psum_tensor([tt, P], F32))

        # per-slot loads +16; per-stream-slot stores +16; compute sems +1
        s_in = [
            ctx.enter_context(nc.semaphore(f"s_in{i}")) for i in range(nslots)
        ]
        s_souts = [
            ctx.enter_context(nc.semaphore(f"s_souts{i}"))
            for i in range(max(nstream, 1))
        ]
        s_soutc = ctx.enter_context(nc.semaphore("s_soutc"))  # cached stores
        s_eye = ctx.enter_context(nc.semaphore("s_eye"))
        s_red = ctx.enter_context(nc.semaphore("s_red"))
        s_sqrt = ctx.enter_context(nc.semaphore("s_sqrt"))
        s_rcp = ctx.enter_context(nc.semaphore("s_rcp"))
        s_tp = ctx.enter_context(nc.semaphore("s_tp"))
        s_ptc = ctx.enter_context(nc.semaphore("s_ptc"))
        s_ccin = ctx.enter_context(nc.semaphore("s_ccin"))
        s_cc = ctx.enter_context(nc.semaphore("s_cc"))
        s_cs = ctx.enter_context(nc.semaphore("s_cs"))
        s_stt = ctx.enter_context(nc.semaphore("s_stt"))
        block = ctx.enter_context(nc.Block())

        def item_src(i):
            t, h = items[i]
            return mx_v[t, :, h]

        def item_dst(i):
            t, h = items[i]
            return out_v[t, :, h]

        @block.gpsimd
        def _(g):
            # pass 1 loads
            for i in range(ni):
                if i in in_val2 and i >= nstream:
                    g.wait_ge(s_red, i - nstream + 1)  # slot's rowsum done
                g.dma_start(slots[slot_of(i)][:, :], item_src(i)).then_inc(
                    s_in[slot_of(i)], 16
                )

            # prefetch the first pass-2 stream loads (fills the AG window)
            if stream_items:
                g.wait_ge(s_red, len(stream_items))  # stream slots all free
            for i in stream_items[:nstream]:
                g.dma_start(slots[slot_of(i)][:, :], item_src(i)).then_inc(
                    s_in[slot_of(i)], 16
                )

            g.wait_ge(s_ccin, 16)  # SP wrote local r_inv to DRAM
            g.collective_compute(
                "AllGather",
                mybir.AluOpType.bypass,
                replica_groups=[list(range(ncores))],
                ins=[cc_in[:]],
                outs=[cc_out[:]],
            ).then_inc(s_cc, 1)

            # remaining pass-2 stream loads (slot free when its store landed)
            for i in stream_items[nstream:]:
                g.wait_ge(s_souts[slot_of(i)], souts_val[i] - 16)
                g.dma_start(slots[slot_of(i)][:, :], item_src(i)).then_inc(
                    s_in[slot_of(i)], 16
                )

        @block.sync
        def _(sp):
            # identity for the PE transpose
            sp.dma_start(eye_sb[:, :], eye[:, :]).then_inc(s_eye, 16)
            # local r_inv (transposed via PE, staged to SBUF) -> DRAM
            sp.wait_ge(s_ptc, 1)
            sp.dma_start(cc_in[:], ptc[:, :]).then_inc(s_ccin, 16)
            # column-scale broadcast once the allgather lands
            sp.wait_ge(s_cc, 1)
            sp.dma_start(
                colscale[:, :], cc_out[:].partition_broadcast(P)
            ).then_inc(s_cs, 16)
            # stores, in pass-2 processing order
            for k, i in enumerate(order):
                sp.wait_ge(s_stt, k + 1)
                if i in in_val2:  # streamed
                    if souts_val[i] > 16:
                        sp.wait_ge(s_souts[slot_of(i)], souts_val[i] - 16)
                    sem = s_souts[slot_of(i)]
                else:
                    sem = s_soutc
                sp.dma_start(item_dst(i), slots[slot_of(i)][:, :]).then_inc(
                    sem, 16
                )
            # all stores landed before halt
            for s_idx in range(nstream):
                sp.wait_ge(s_souts[s_idx], souts_count[s_idx])
            if ncache:
                sp.wait_ge(s_soutc, 16 * ncache)

        @block.scalar
        def _(s):
            # pass 1: rowsums via in-place Copy with free-axis accumulate
            for i in range(ni):
                s.wait_ge(s_in[slot_of(i)], in_val1[i])
                s.activation(
                    slots[slot_of(i)][:, :],
                    slots[slot_of(i)][:, :],
                    mybir.ActivationFunctionType.Copy,
                    accum_out=ps[:, i : i + 1],
                ).then_inc(s_red, 1)
            # sqrt after the DVE half-combine
            s.wait_ge(s_red, ni + 1)
            s.sqrt(rinv[:, :], rs[:, :]).then_inc(s_sqrt, 1)

        @block.tensor
        def _(pe):
            # r_inv [128, tt] -> [tt, 128] in PSUM (via identity)
            pe.wait_ge(s_eye, 16)
            pe.wait_ge(s_rcp, 1)
            pe.transpose(pt[:, :], rinv[:, :], eye_sb[:, :]).then_inc(s_tp, 1)

        @block.vector
        def _(v):
            # combine halves: rs[:, t] = sum_h ps[:, t*H + h]
            v.wait_ge(s_red, ni)
            if H == 2:
                v.scalar_tensor_tensor(
                    rs[:, :],
                    ps[:, 0::2],
                    1.0,
                    ps[:, 1::2],
                    op0=mybir.AluOpType.mult,
                    op1=mybir.AluOpType.add,
                ).then_inc(s_red, 1)
            else:
                raise NotImplementedError
            # r_inv = 1/sqrt(rowsum): ACT does sqrt, finish with reciprocal
            v.wait_ge(s_sqrt, 1)
            v.reciprocal(rinv[:, :], rinv[:, :]).then_inc(s_rcp, 1)
            # stage the PE-transposed r_inv out of PSUM for the DRAM write
            v.wait_ge(s_tp, 1)
            v.tensor_copy(ptc[:, :], pt[:, :]).then_inc(s_ptc, 1)
            # pass 2: fused row+column scale, in place
            # (self-wait drains the reciprocal writeback before stts read rinv)
            v.wait_ge(s_rcp, 1)
            v.wait_ge(s_cs, 16)
            for i in order:
                t, h = items[i]
                if i in in_val2:  # streamed: wait for its pass-2 load
                    v.wait_ge(s_in[slot_of(i)], in_val2[i])
                v.scalar_tensor_tensor(
                    slots[slot_of(i)][:, :],
                    slots[slot_of(i)][:, :],
                    rinv[:, t : t + 1],
                    colscale[:, h * w : (h + 1) * w],
                    op0=mybir.AluOpType.mult,
                    op1=mybir.AluOpType.mult,
                ).then_inc(s_stt, 1)

    return nc


_NC_CACHE = {}


def _get_nc(n=N, ncores=NCORES):
    key = (n, ncores)
    if key not in _NC_CACHE:
        _NC_CACHE[key] = build_kernel(n, ncores)
    return _NC_CACHE[key]


def kernel(adj, **run_kwargs):
    adj = np.asarray(adj)
    assert adj.shape == (N, N) and adj.dtype == np.float32
    mx = adj.copy()
    idx = np.arange(N)
    mx[idx, idx] += 1.0
    eye = np.eye(P, dtype=np.float32)

    in_maps = [
        {"mx": mx[c * SHARD : (c + 1) * SHARD], "eye": eye}
        for c in range(NCORES)
    ]
    nc = _get_nc()
    res = run_bass_kernel_spmd(nc, in_maps, list(range(NCORES)), **run_kwargs)
    out = np.concatenate([res.results[c]["out"] for c in range(NCORES)], axis=0)
    if run_kwargs:
        return out, res
    return out


# revision 18
# speedup vs baseline: 1.3551x; 1.0315x over previous
"""Normalized-adjacency kernel (EstimateAdj.normalize, symmetric=False) for TRN2.

out = mx * r_inv[:, None] * r_inv[None, :]   where mx = adj + I,
r_inv = rowsum(mx) ** -0.5.

Strategy (8 NeuronCores, row-sharded, raw Bass with explicit semaphores):
  - host: add 1.0 to the diagonal (O(n)), split rows into 8 shards
  - device, per core: work items are HALF-tiles [128 x n/2]
    (tile t = shard rows [t*128:(t+1)*128], halves h split the columns):
      pass 1: stream the first 11 halves through 5 SBUF slots, keep the last
              5 halves resident.  Rowsums run on the SCALAR engine
              (activation Copy with accum_out), so the DVE stays free and the
              loads, not the reduces, pace the pass.
      r_inv = 1/sqrt(rowsum) (ACT sqrt + DVE reciprocal); PE transposes
      r_inv via an identity matmul so the DRAM write of the local r_inv is
      8 contiguous 512B descriptors instead of 128 scattered 32B ones.
      AllGather local r_inv (DRAM) -> full n vector; while it is in flight
      the 5 stream slots prefetch the first 5 pass-2 halves (~10 MiB).
      pass 2: fused in-place DVE scalar_tensor_tensor per half:
              half = (half * r_inv_row_scalar) * colscale[:, h-slice]; store.
              Prefetched stream halves are processed FIRST so their stores
              complete early and un-gate the remaining reloads (the reload
              chain is bandwidth-bound, not latency-bound).
  - engines: gpsimd/Pool = loads + allgather; SP/sync = stores + small DMAs;
    DVE = fused scales; ACT = rowsums + sqrt; PE = r_inv transpose.
  - host: concatenate the 8 output shards
"""

from contextlib import ExitStack

import numpy as np

import concourse.bass as bass
import concourse.mybir as mybir
from concourse.bass_utils import run_bass_kernel_spmd

N = 8192
NCORES = 8
SHARD = N // NCORES  # 1024
P = 128
T = SHARD // P  # 8 tiles per core
H = 2  # column halves per tile

F32 = mybir.dt.float32
NSTREAM = 6  # streaming half-tile slots
NCACHE = 4  # pass-1-resident half-tile slots


def build_kernel(n=N, ncores=NCORES):
    shard = n // ncores
    tt = shard // P
    w = n // H  # half width
    items = [(t, h) for t in range(tt) for h in range(H)]  # load order
    ni = len(items)

    ncache = min(NCACHE, max(ni - NSTREAM, 0))
    nstream = min(NSTREAM, ni - ncache)
    stream_items = list(range(ni - ncache))  # indices into `items`
    cached_items = list(range(ni - ncache, ni))

    def slot_of(i):
        if i >= ni - ncache:
            return nstream + (i - (ni - ncache))
        return i % nstream

    # pass-2 order: prefetched stream halves first (their stores un-gate the
    # reloads), then cached halves, then the reloaded stream halves.
    order = (
        stream_items[:nstream] + cached_items + stream_items[nstream:]
    )

    # per-slot cumulative load-completion values (s_in[slot])
    nslots = nstream + ncache
    in_count = [0] * nslots
    in_val1 = [0] * ni
    for i in range(ni):
        in_count[slot_of(i)] += 16
        in_val1[i] = in_count[slot_of(i)]
    in_val2 = {}
    for i in stream_items:
        in_count[slot_of(i)] += 16
        in_val2[i] = in_count[slot_of(i)]

    # per-stream-slot cumulative store-completion values (s_souts[slot])
    souts_count = [0] * max(nstream, 1)
    souts_val = {}
    for i in stream_items:
        souts_count[slot_of(i)] += 16
        souts_val[i] = souts_count[slot_of(i)]

    nc = bass.Bass(num_devices=ncores)
    mx = nc.dram_tensor("mx", [shard, n], F32, kind="ExternalInput")
    eye = nc.dram_tensor("eye", [P, P], F32, kind="ExternalInput")
    out = nc.dram_tensor("out", [shard, n], F32, kind="ExternalOutput")
    cc_in = nc.dram_tensor("cc_in", [shard], F32)
    cc_out = nc.dram_tensor("cc_out", [n], F32, addr_space="Shared")

    # blocked tiling: tile t, partition p, half h -> shard row t*128 + p
    mx_v = mx.rearrange("(t p) (h w) -> t p h w", p=P, h=H)
    out_v = out.rearrange("(t p) (h w) -> t p h w", p=P, h=H)

    with ExitStack() as ctx:
        slots = [
            ctx.enter_context(nc.sbuf_tensor(f"tile{i}", [P, w], F32))
            for i in range(nslots)
        ]
        colscale = ctx.enter_context(nc.sbuf_tensor("colscale", [P, n], F32))
        eye_sb = ctx.enter_context(nc.sbuf_tensor("eye_sb", [P, P], F32))
        ps = ctx.enter_context(nc.sbuf_tensor("ps", [P, ni], F32))
        rs = ctx.enter_context(nc.sbuf_tensor("rs", [P, tt], F32))
        rinv = ctx.enter_context(nc.sbuf_tensor("rinv", [P, tt], F32))
        ptc = ctx.enter_context(nc.sbuf_tensor("ptc", [tt, P], F32))
        pt = ctx.enter_context(nc.psum_tensor([tt, P], F32))

        # per-slot loads +16; per-stream-slot stores +16; compute sems +1
        s_in = [
            ctx.enter_context(nc.semaphore(f"s_in{i}")) for i in range(nslots)
        ]
        s_souts = [
            ctx.enter_context(nc.semaphore(f"s_souts{i}"))
            for i in range(max(nstream, 1))
        ]
        s_soutc = ctx.enter_context(nc.semaphore("s_soutc"))  # cached stores
        s_eye = ctx.enter_context(nc.semaphore("s_eye"))
        s_red = ctx.enter_context(nc.semaphore("s_red"))
        s_sqrt = ctx.enter_context(nc.semaphore("s_sqrt"))
        s_rcp = ctx.enter_context(nc.semaphore("s_rcp"))
        s_tp = ctx.enter_context(nc.semaphore("s_tp"))
        s_ptc = ctx.enter_context(nc.semaphore("s_ptc"))
        s_ccin = ctx.enter_context(nc.semaphore("s_ccin"))
        s_cc = ctx.enter_context(nc.semaphore("s_cc"))
        s_cs = [
            ctx.enter_context(nc.semaphore(f"s_cs{h}")) for h in range(H)
        ]
        s_stt = ctx.enter_context(nc.semaphore("s_stt"))
        block = ctx.enter_context(nc.Block())

        def item_src(i):
            t, h = items[i]
            return mx_v[t, :, h]

        def item_dst(i):
            t, h = items[i]
            return out_v[t, :, h]

        @block.gpsimd
        def _(g):
            # pass 1 loads
            for i in range(ni):
                if i in in_val2 and i >= nstream:
                    g.wait_ge(s_red, i - nstream + 1)  # slot's rowsum done
                g.dma_start(slots[slot_of(i)][:, :], item_src(i)).then_inc(
                    s_in[slot_of(i)], 16
                )

            # prefetch the first pass-2 stream loads (fills the AG window)
            if stream_items:
                g.wait_ge(s_red, len(stream_items))  # stream slots all free
            for i in stream_items[:nstream]:
                g.dma_start(slots[slot_of(i)][:, :], item_src(i)).then_inc(
                    s_in[slot_of(i)], 16
                )

            g.wait_ge(s_ccin, 16)  # SP wrote local r_inv to DRAM
            g.collective_compute(
                "AllGather",
                mybir.AluOpType.bypass,
                replica_groups=[list(range(ncores))],
                ins=[cc_in[:]],
                outs=[cc_out[:]],
            ).then_inc(s_cc, 1)

            # remaining pass-2 stream loads (slot free when its store landed)
            for i in stream_items[nstream:]:
                g.wait_ge(s_souts[slot_of(i)], souts_val[i] - 16)
                g.dma_start(slots[slot_of(i)][:, :], item_src(i)).then_inc(
                    s_in[slot_of(i)], 16
                )

        @block.sync
        def _(sp):
            # identity for the PE transpose
            sp.dma_start(eye_sb[:, :], eye[:, :]).then_inc(s_eye, 16)
            # local r_inv (transposed via PE, staged to SBUF) -> DRAM
            sp.wait_ge(s_ptc, 1)
            sp.dma_start(cc_in[:], ptc[:, :]).then_inc(s_ccin, 16)
            # column-scale broadcast once the allgather lands, one chunk per
            # column-half so the first scales start after w columns, not n
            sp.wait_ge(s_cc, 1)
            for h in range(H):
                sp.dma_start(
                    colscale[:, h * w : (h + 1) * w],
                    cc_out[h * w : (h + 1) * w].partition_broadcast(P),
                ).then_inc(s_cs[h], 16)
            # stores, in pass-2 processing order
            for k, i in enumerate(order):
                sp.wait_ge(s_stt, k + 1)
                if i in in_val2:  # streamed
                    if souts_val[i] > 16:
                        sp.wait_ge(s_souts[slot_of(i)], souts_val[i] - 16)
                    sem = s_souts[slot_of(i)]
                else:
                    sem = s_soutc
                sp.dma_start(item_dst(i), slots[slot_of(i)][:, :]).then_inc(
                    sem, 16
                )
            # all stores landed before halt
            for s_idx in range(nstream):
                sp.wait_ge(s_souts[s_idx], souts_count[s_idx])
            if ncache:
                sp.wait_ge(s_soutc, 16 * ncache)

        @block.scalar
        def _(s):
            # pass 1: rowsums via in-place Copy with free-axis accumulate
            for i in range(ni):
                s.wait_ge(s_in[slot_of(i)], in_val1[i])
                s.activation(
                    slots[slot_of(i)][:, :],
                    slots[slot_of(i)][:, :],
                    mybir.ActivationFunctionType.Copy,
                    accum_out=ps[:, i : i + 1],
                ).then_inc(s_red, 1)
            # sqrt after the DVE half-combine
            s.wait_ge(s_red, ni + 1)
            s.sqrt(rinv[:, :], rs[:, :]).then_inc(s_sqrt, 1)

        @block.tensor
        def _(pe):
            # r_inv [128, tt] -> [tt, 128] in PSUM (via identity)
            pe.wait_ge(s_eye, 16)
            pe.wait_ge(s_rcp, 1)
            pe.transpose(pt[:, :], rinv[:, :], eye_sb[:, :]).then_inc(s_tp, 1)

        @block.vector
        def _(v):
            # combine halves: rs[:, t] = sum_h ps[:, t*H + h]
            v.wait_ge(s_red, ni)
            if H == 2:
                v.scalar_tensor_tensor(
                    rs[:, :],
                    ps[:, 0::2],
                    1.0,
                    ps[:, 1::2],
                    op0=mybir.AluOpType.mult,
                    op1=mybir.AluOpType.add,
                ).then_inc(s_red, 1)
            else:
                raise NotImplementedError
            # r_inv = 1/sqrt(rowsum): ACT does sqrt, finish with reciprocal
            v.wait_ge(s_sqrt, 1)
            v.reciprocal(rinv[:, :], rinv[:, :]).then_inc(s_rcp, 1)
            # stage the PE-transposed r_inv out of PSUM for the DRAM write
            v.wait_ge(s_tp, 1)
            v.tensor_copy(ptc[:, :], pt[:, :]).then_inc(s_ptc, 1)
            # pass 2: fused row+column scale, in place
            # (self-wait drains the reciprocal writeback before stts read rinv)
            v.wait_ge(s_rcp, 1)
            cs_seen = set()
            for i in order:
                t, h = items[i]
                if h not in cs_seen:
                    cs_seen.add(h)
                    v.wait_ge(s_cs[h], 16)
                if i in in_val2:  # streamed: wait for its pass-2 load
                    v.wait_ge(s_in[slot_of(i)], in_val2[i])
                v.scalar_tensor_tensor(
                    slots[slot_of(i)][:, :],
                    slots[slot_of(i)][:, :],
                    rinv[:, t : t + 1],
                    colscale[:, h * w : (h + 1) * w],
                    op0=mybir.AluOpType.mult,
                    op1=mybir.AluOpType.mult,
                ).then_inc(s_stt, 1)

    return nc


_NC_CACHE = {}


def _get_nc(n=N, ncores=NCORES):
    key = (n, ncores)
    if key not in _NC_CACHE:
        _NC_CACHE[key] = build_kernel(n, ncores)
    return _NC_CACHE[key]


def kernel(adj, **run_kwargs):
    adj = np.asarray(adj)
    assert adj.shape == (N, N) and adj.dtype == np.float32
    mx = adj.copy()
    idx = np.arange(N)
    mx[idx, idx] += 1.0
    eye = np.eye(P, dtype=np.float32)

    in_maps = [
        {"mx": mx[c * SHARD : (c + 1) * SHARD], "eye": eye}
        for c in range(NCORES)
    ]
    nc = _get_nc()
    try:
        res = run_bass_kernel_spmd(nc, in_maps, list(range(NCORES)), **run_kwargs)
    except Exception:
        # transient device hiccups (e.g. a wedged core from an earlier
        # process) sometimes clear on a second attempt
        import time

        time.sleep(2.0)
        res = run_bass_kernel_spmd(nc, in_maps, list(range(NCORES)), **run_kwargs)
    out = np.concatenate([res.results[c]["out"] for c in range(NCORES)], axis=0)
    if run_kwargs:
        return out, res
    return out


# revision 21
# speedup vs baseline: 1.3598x; 1.0035x over previous
"""Normalized-adjacency kernel (EstimateAdj.normalize, symmetric=False) for TRN2.

out = mx * r_inv[:, None] * r_inv[None, :]   where mx = adj + I,
r_inv = rowsum(mx) ** -0.5.

Strategy (8 NeuronCores, row-sharded, raw Bass with explicit semaphores):
  - host: add 1.0 to the diagonal (O(n)), split rows into 8 shards
  - device, per core: work items are HALF-tiles [128 x n/2]
    (tile t = shard rows [t*128:(t+1)*128], halves h split the columns):
      pass 1: stream the first 11 halves through 5 SBUF slots, keep the last
              5 halves resident.  Rowsums run on the SCALAR engine
              (activation Copy with accum_out), so the DVE stays free and the
              loads, not the reduces, pace the pass.
      r_inv = 1/sqrt(rowsum) (ACT sqrt + DVE reciprocal); PE transposes
      r_inv via an identity matmul so the DRAM write of the local r_inv is
      8 contiguous 512B descriptors instead of 128 scattered 32B ones.
      AllGather local r_inv (DRAM) -> full n vector; while it is in flight
      the 5 stream slots prefetch the first 5 pass-2 halves (~10 MiB).
      pass 2: fused in-place DVE scalar_tensor_tensor per half:
              half = (half * r_inv_row_scalar) * colscale[:, h-slice]; store.
              Prefetched stream halves are processed FIRST so their stores
              complete early and un-gate the remaining reloads (the reload
              chain is bandwidth-bound, not latency-bound).
  - engines: gpsimd/Pool = loads + allgather; SP/sync = stores + small DMAs;
    DVE = fused scales; ACT = rowsums + sqrt; PE = r_inv transpose.
  - host: concatenate the 8 output shards
"""

from contextlib import ExitStack

import numpy as np

import concourse.bass as bass
import concourse.mybir as mybir
from concourse.bass_utils import run_bass_kernel_spmd

N = 8192
NCORES = 8
SHARD = N // NCORES  # 1024
P = 128
T = SHARD // P  # 8 tiles per core
H = 2  # column halves per tile

F32 = mybir.dt.float32
NSTREAM = 6  # streaming half-tile slots
NCACHE = 4  # pass-1-resident half-tile slots


def build_kernel(n=N, ncores=NCORES):
    shard = n // ncores
    tt = shard // P
    w = n // H  # half width
    items = [(t, h) for t in range(tt) for h in range(H)]  # load order
    ni = len(items)

    ncache = min(NCACHE, max(ni - NSTREAM, 0))
    nstream = min(NSTREAM, ni - ncache)
    stream_items = list(range(ni - ncache))  # indices into `items`
    cached_items = list(range(ni - ncache, ni))

    def slot_of(i):
        if i >= ni - ncache:
            return nstream + (i - (ni - ncache))
        return i % nstream

    # pass-2 order: prefetched stream halves first (their stores un-gate the
    # reloads), then cached halves, then the reloaded stream halves.
    order = (
        stream_items[:nstream] + cached_items + stream_items[nstream:]
    )

    # per-slot cumulative load-completion values (s_in[slot])
    nslots = nstream + ncache
    in_count = [0] * nslots
    in_val1 = [0] * ni
    for i in range(ni):
        in_count[slot_of(i)] += 16
        in_val1[i] = in_count[slot_of(i)]
    in_val2 = {}
    for i in stream_items:
        in_count[slot_of(i)] += 16
        in_val2[i] = in_count[slot_of(i)]

    # per-stream-slot cumulative store-completion values (s_souts[slot])
    souts_count = [0] * max(nstream, 1)
    souts_val = {}
    for i in stream_items:
        souts_count[slot_of(i)] += 16
        souts_val[i] = souts_count[slot_of(i)]

    # rowsum -> r_inv -> transpose -> DRAM chain is pipelined in two groups
    # (all-but-last tile early, last tile late) so most of it hides under the
    # tail of pass 1
    groups = [(0, tt - 1), (tt - 1, tt)] if tt >= 2 else [(0, tt)]
    ng = len(groups)

    nc = bass.Bass(num_devices=ncores)
    mx = nc.dram_tensor("mx", [shard, n], F32, kind="ExternalInput")
    eye = nc.dram_tensor("eye", [P, P], F32, kind="ExternalInput")
    out = nc.dram_tensor("out", [shard, n], F32, kind="ExternalOutput")
    cc_in = nc.dram_tensor("cc_in", [shard], F32)
    cc_out = nc.dram_tensor("cc_out", [n], F32, addr_space="Shared")

    # blocked tiling: tile t, partition p, half h -> shard row t*128 + p
    mx_v = mx.rearrange("(t p) (h w) -> t p h w", p=P, h=H)
    out_v = out.rearrange("(t p) (h w) -> t p h w", p=P, h=H)

    with ExitStack() as ctx:
        slots = [
            ctx.enter_context(nc.sbuf_tensor(f"tile{i}", [P, w], F32))
            for i in range(nslots)
        ]
        colscale = ctx.enter_context(nc.sbuf_tensor("colscale", [P, n], F32))
        eye_sb = ctx.enter_context(nc.sbuf_tensor("eye_sb", [P, P], F32))
        ps = ctx.enter_context(nc.sbuf_tensor("ps", [P, ni], F32))
        rs = ctx.enter_context(nc.sbuf_tensor("rs", [P, tt], F32))
        rinv = ctx.enter_context(nc.sbuf_tensor("rinv", [P, tt], F32))
        ptc = [
            ctx.enter_context(nc.sbuf_tensor(f"ptc{g}", [b - a, P], F32))
            for g, (a, b) in enumerate(groups)
        ]
        pt = [
            ctx.enter_context(nc.psum_tensor(f"pt{g}", [b - a, P], F32))
            for g, (a, b) in enumerate(groups)
        ]

        # per-slot loads +16; per-stream-slot stores +16; compute sems +1
        s_in = [
            ctx.enter_context(nc.semaphore(f"s_in{i}")) for i in range(nslots)
        ]
        s_souts = [
            ctx.enter_context(nc.semaphore(f"s_souts{i}"))
            for i in range(max(nstream, 1))
        ]
        s_soutc = ctx.enter_context(nc.semaphore("s_soutc"))  # cached stores
        s_eye = ctx.enter_context(nc.semaphore("s_eye"))
        s_red = ctx.enter_context(nc.semaphore("s_red"))
        s_cmb = [
            ctx.enter_context(nc.semaphore(f"s_cmb{g}")) for g in range(ng)
        ]
        s_sqrt = [
            ctx.enter_context(nc.semaphore(f"s_sqrt{g}")) for g in range(ng)
        ]
        s_rcp = ctx.enter_context(nc.semaphore("s_rcp"))
        s_tp = [
            ctx.enter_context(nc.semaphore(f"s_tp{g}")) for g in range(ng)
        ]
        s_ptc = [
            ctx.enter_context(nc.semaphore(f"s_ptc{g}")) for g in range(ng)
        ]
        s_ccin = ctx.enter_context(nc.semaphore("s_ccin"))
        s_cc = ctx.enter_context(nc.semaphore("s_cc"))
        NCS = 2 * H  # column-scale broadcast chunks (quarters)
        w2 = n // NCS
        s_cs = [
            ctx.enter_context(nc.semaphore(f"s_cs{q}")) for q in range(NCS)
        ]
        s_stt = ctx.enter_context(nc.semaphore("s_stt"))
        block = ctx.enter_context(nc.Block())

        def item_src(i):
            t, h = items[i]
            return mx_v[t, :, h]

        def item_dst(i):
            t, h = items[i]
            return out_v[t, :, h]

        @block.gpsimd
        def _(g):
            # pass 1 loads
            for i in range(ni):
                if i in in_val2 and i >= nstream:
                    g.wait_ge(s_red, i - nstream + 1)  # slot's rowsum done
                g.dma_start(slots[slot_of(i)][:, :], item_src(i)).then_inc(
                    s_in[slot_of(i)], 16
                )

            # prefetch the first pass-2 stream loads (fills the AG window)
            if stream_items:
                g.wait_ge(s_red, len(stream_items))  # stream slots all free
            for i in stream_items[:nstream]:
                g.dma_start(slots[slot_of(i)][:, :], item_src(i)).then_inc(
                    s_in[slot_of(i)], 16
                )

            g.wait_ge(s_ccin, 16 * ng)  # SP wrote local r_inv to DRAM
            g.collective_compute(
                "AllGather",
                mybir.AluOpType.bypass,
                replica_groups=[list(range(ncores))],
                ins=[cc_in[:]],
                outs=[cc_out[:]],
            ).then_inc(s_cc, 1)

            # remaining pass-2 stream loads (slot free when its store landed)
            for i in stream_items[nstream:]:
                g.wait_ge(s_souts[slot_of(i)], souts_val[i] - 16)
                g.dma_start(slots[slot_of(i)][:, :], item_src(i)).then_inc(
                    s_in[slot_of(i)], 16
                )

        @block.sync
        def _(sp):
            # identity for the PE transpose
            sp.dma_start(eye_sb[:, :], eye[:, :]).then_inc(s_eye, 16)
            # local r_inv (transposed via PE, staged to SBUF) -> DRAM
            for g, (a, b) in enumerate(groups):
                sp.wait_ge(s_ptc[g], 1)
                sp.dma_start(
                    cc_in[a * P : b * P], ptc[g][:, :]
                ).then_inc(s_ccin, 16)
            # column-scale broadcast once the allgather lands, one quarter
            # chunk at a time so the first scales start as soon as possible
            sp.wait_ge(s_cc, 1)
            for q in range(NCS):
                sp.dma_start(
                    colscale[:, q * w2 : (q + 1) * w2],
                    cc_out[q * w2 : (q + 1) * w2].partition_broadcast(P),
                ).then_inc(s_cs[q], 16)
            # stores, in pass-2 processing order
            for k, i in enumerate(order):
                sp.wait_ge(s_stt, k + 1)
                if i in in_val2:  # streamed
                    if souts_val[i] > 16:
                        sp.wait_ge(s_souts[slot_of(i)], souts_val[i] - 16)
                    sem = s_souts[slot_of(i)]
                else:
                    sem = s_soutc
                sp.dma_start(item_dst(i), slots[slot_of(i)][:, :]).then_inc(
                    sem, 16
                )
            # all stores landed before halt
            for s_idx in range(nstream):
                sp.wait_ge(s_souts[s_idx], souts_count[s_idx])
            if ncache:
                sp.wait_ge(s_soutc, 16 * ncache)

        @block.scalar
        def _(s):
            # pass 1: rowsums via in-place Copy with free-axis accumulate.
            # Group sqrts (in place on rs) are interleaved: group g's sqrt is
            # emitted right after the copies it depends on, so early groups'
            # sqrt runs in the gaps while later copies wait on their loads.
            done = 0
            for g, (a, b) in enumerate(groups):
                for i in range(done, b * H):
                    s.wait_ge(s_in[slot_of(i)], in_val1[i])
                    s.activation(
                        slots[slot_of(i)][:, :],
                        slots[slot_of(i)][:, :],
                        mybir.ActivationFunctionType.Copy,
                        accum_out=ps[:, i : i + 1],
                    ).then_inc(s_red, 1)
                done = b * H
                s.wait_ge(s_cmb[g], 1)
                s.sqrt(rs[:, a:b], rs[:, a:b]).then_inc(s_sqrt[g], 1)

        @block.tensor
        def _(pe):
            # sqrt(rowsum) [128, g] -> [g, 128] in PSUM (via identity)
            pe.wait_ge(s_eye, 16)
            for g, (a, b) in enumerate(groups):
                pe.wait_ge(s_sqrt[g], 1)
                pe.transpose(
                    pt[g][:, :], rs[:, a:b], eye_sb[:, :]
                ).then_inc(s_tp[g], 1)

        @block.vector
        def _(v):
            assert H == 2
            for g, (a, b) in enumerate(groups):
                # combine halves: rs[:, t] = sum_h ps[:, t*H + h]
                v.wait_ge(s_red, b * H)
                v.scalar_tensor_tensor(
                    rs[:, a:b],
                    ps[:, 2 * a : 2 * b : 2],
                    1.0,
                    ps[:, 2 * a + 1 : 2 * b : 2],
                    op0=mybir.AluOpType.mult,
                    op1=mybir.AluOpType.add,
                ).then_inc(s_cmb[g], 1)
                # row-scalar r_inv for the pass-2 scales
                v.wait_ge(s_sqrt[g], 1)
                v.reciprocal(rinv[:, a:b], rs[:, a:b]).then_inc(s_rcp, 1)
                # r_inv (transposed) = 1/transpose(sqrt): one fused step out
                # of PSUM, ready for the DRAM write
                v.wait_ge(s_tp[g], 1)
                v.reciprocal(ptc[g][:, :], pt[g][:, :]).then_inc(s_ptc[g], 1)
            # pass 2: fused row+column scale, in place
            # (self-wait drains the reciprocal writebacks before stts)
            v.wait_ge(s_rcp, ng)
            cs_seen = set()
            for i in order:
                t, h = items[i]
                for q in (2 * h, 2 * h + 1):
                    if q not in cs_seen:
                        cs_seen.add(q)
                        v.wait_ge(s_cs[q], 16)
                if i in in_val2:  # streamed: wait for its pass-2 load
                    v.wait_ge(s_in[slot_of(i)], in_val2[i])
                v.scalar_tensor_tensor(
                    slots[slot_of(i)][:, :],
                    slots[slot_of(i)][:, :],
                    rinv[:, t : t + 1],
                    colscale[:, h * w : (h + 1) * w],
                    op0=mybir.AluOpType.mult,
                    op1=mybir.AluOpType.mult,
                ).then_inc(s_stt, 1)

    return nc


_NC_CACHE = {}


def _get_nc(n=N, ncores=NCORES):
    key = (n, ncores)
    if key not in _NC_CACHE:
        _NC_CACHE[key] = build_kernel(n, ncores)
    return _NC_CACHE[key]


def kernel(adj, **run_kwargs):
    adj = np.asarray(adj)
    assert adj.shape == (N, N) and adj.dtype == np.float32
    mx = adj.copy()
    idx = np.arange(N)
    mx[idx, idx] += 1.0
    eye = np.eye(P, dtype=np.float32)

    in_maps = [
        {"mx": mx[c * SHARD : (c + 1) * SHARD], "eye": eye}
        for c in range(NCORES)
    ]
    nc = _get_nc()
    try:
        res = run_bass_kernel_spmd(nc, in_maps, list(range(NCORES)), **run_kwargs)
    except Exception:
        # transient device hiccups (e.g. a wedged core from an earlier
        # process) sometimes clear on a second attempt
        import time

        time.sleep(2.0)
        res = run_bass_kernel_spmd(nc, in_maps, list(range(NCORES)), **run_kwargs)
    out = np.concatenate([res.results[c]["out"] for c in range(NCORES)], axis=0)
    if run_kwargs:
        return out, res
    return out
